# revision 1
# baseline (speedup 1.0000x reference)
"""GATv2 encoder (2-layer, PyG GATv2Conv semantics) on 8 TRN2 NeuronCores.

Sharding: dst-node blocks of 6250 nodes per core; edges live with their dst
core so segment softmax/aggregation are local; one AllGather of the folded
source-side node table between layers.

Algorithm (host-validated against the jax reference to ~5e-6 rel err):
- |att| folded into Wl/Wr columns, columns permuted pos-att-first per head.
  Per-edge logits become  sum_pos lrelu(u) - sum_neg lrelu(u)  with
  u = ul[src] + ur[dst] gathered directly from folded tables (second gather
  accumulates via the DMA CCE-add path).
- Segment softmax skips max-subtraction (|logits| <= ~1 for this model).
- sum_e alpha*(ul+ur) = sum_e alpha*ul + ur, so the same u tiles feed the
  aggregation; per-chunk one-hot matmul accumulates [num | den] in PSUM.
"""
import numpy as np

try:
    import concourse  # noqa: F401
except ImportError:  # pragma: no cover
    import sys
    sys.path.insert(0, "/opt/trn_rl_repo")

from concourse import bass, bacc, mybir, tile
from concourse import bass_utils
from concourse.bass import IndirectOffsetOnAxis

F32 = mybir.dt.float32
I32 = mybir.dt.int32

N_NODES = 50000
N_CORES = 8
FEAT = 128
HEADS1 = 4


class Cfg:
    def __init__(self, n_nodes, n_cores, feat, heads1, T, dtype=F32):
        self.N = n_nodes
        self.NC = n_cores
        self.NPC = n_nodes // n_cores
        self.P = 128
        self.CHUNKS = (self.NPC + 127) // 128
        self.SLOTS = self.CHUNKS * 128
        self.F = feat
        self.H1 = heads1
        self.T = T
        self.TD = dtype


# ---------------------------------------------------------------- host prep

def prep_weights(att, Wl, bl, Wr, br, bias):
    H, C = att.shape
    a = att.reshape(-1).astype(np.float64)
    perm, pos_counts = [], []
    for h in range(H):
        cols = np.arange(h * C, (h + 1) * C)
        pos = cols[a[cols] >= 0]
        neg = cols[a[cols] < 0]
        perm.extend(pos.tolist() + neg.tolist())
        pos_counts.append(len(pos))
    perm = np.array(perm, dtype=np.int64)
    absa = np.maximum(np.abs(a[perm]), 1e-12)
    return dict(
        perm=perm, pos_counts=pos_counts,
        Wl=(Wl[:, perm] * absa[None, :]).astype(np.float32),
        bl=(bl[perm] * absa).astype(np.float32),
        Wr=(Wr[:, perm] * absa[None, :]).astype(np.float32),
        br=(br[perm] * absa).astype(np.float32),
        inva=(1.0 / absa).astype(np.float32),
        bias=bias[perm].astype(np.float32),
    )


def prep_graph(edge_index, cfg, T_override=None):
    """Per-core chunked edge layout for dma_gather (int16 indices).

    Edges of each chunk are ordered [src<32768 section | src>=32768 section],
    each section padded to a global fixed tile count (T_LO / T_HI).  Gather
    index arrays are int16, wrapped in 16 partitions (column-major groups of
    16) and replicated 8x down the partition dim as the HW requires.
    Pads: src->row 0 of its half-table, dst-table->SLOTS (zeroed dummy row),
    slot->999 (no one-hot match), node_ids pad->SLOTS+8.
    """
    import heapq
    N, NPC, P, CHUNKS = cfg.N, cfg.NPC, cfg.P, cfg.CHUNKS
    HALF = 32768
    src = np.asarray(edge_index[0], dtype=np.int64)
    dst = np.asarray(edge_index[1], dtype=np.int64)
    loops = np.arange(N, dtype=np.int64)
    src = np.concatenate([src, loops])
    dst = np.concatenate([dst, loops])

    cores = []
    maxTlo = maxThi = 0
    for c in range(cfg.NC):
        lo = c * NPC
        m = (dst >= lo) & (dst < lo + NPC)
        s_c = src[m]
        d_c = dst[m] - lo
        deg = np.bincount(d_c, minlength=NPC)
        order = np.argsort(-deg, kind="stable")
        heap = [(0, g) for g in range(CHUNKS)]
        heapq.heapify(heap)
        bin_nodes = [[] for _ in range(CHUNKS)]
        bin_sum = [0] * CHUNKS
        for n in order:
            while True:
                sm, g = heapq.heappop(heap)
                if len(bin_nodes[g]) < P:
                    break
            bin_nodes[g].append(int(n))
            bin_sum[g] = sm + int(deg[n])
            if len(bin_nodes[g]) < P:
                heapq.heappush(heap, (bin_sum[g], g))
        eorder = np.argsort(d_c, kind="stable")
        starts = np.zeros(NPC + 1, dtype=np.int64)
        np.cumsum(deg, out=starts[1:])
        s_sorted = s_c[eorder]
        # per-chunk edge lists split by src half
        chunk_edges = []
        for g in range(CHUNKS):
            lo_s, lo_d, hi_s, hi_d = [], [], [], []
            for slot, n in enumerate(bin_nodes[g]):
                a, b = starts[n], starts[n + 1]
                for s_val in s_sorted[a:b]:
                    if s_val < HALF:
                        lo_s.append(s_val); lo_d.append((n, slot))
                    else:
                        hi_s.append(s_val - HALF); hi_d.append((n, slot))
            maxTlo = max(maxTlo, (len(lo_s) + P - 1) // P)
            maxThi = max(maxThi, (len(hi_s) + P - 1) // P)
            chunk_edges.append((lo_s, lo_d, hi_s, hi_d))
        cores.append((bin_nodes, chunk_edges))

    T_LO = max(maxTlo, 1)
    T_HI = max(maxThi, 1) if N > HALF else maxThi
    T = T_LO + T_HI

    def wrap16(ids):
        # position i -> unwrapped[i]; wrapped[p, s] = ids[s*16 + p]; tile 8x
        a = np.asarray(ids, dtype=np.int16).reshape(-1, 16).T
        return np.tile(a, (8, 1))

    out = []
    for c in range(cfg.NC):
        bin_nodes, chunk_edges = cores[c]
        xlw = np.zeros((CHUNKS, 128, T * 8), dtype=np.int16)
        xrw = np.zeros((CHUNKS, 128, T * 8), dtype=np.int16)
        dstl = np.full((CHUNKS, P, T), 999.0, dtype=np.float32)
        dstlT = np.full((CHUNKS, T * P), 999.0, dtype=np.float32)
        node_ids = np.full((CHUNKS, P), cfg.SLOTS + 8, dtype=np.int32)
        for g in range(CHUNKS):
            lo_s, lo_d, hi_s, hi_d = chunk_edges[g]
            for slot, n in enumerate(bin_nodes[g]):
                node_ids[g, slot] = n
            n_lo, n_hi = T_LO * P, T_HI * P
            ls = np.zeros(n_lo, np.int64); ls[:len(lo_s)] = lo_s
            hs = np.zeros(n_hi, np.int64); hs[:len(hi_s)] = hi_s
            xd = np.full(n_lo + n_hi, cfg.SLOTS, np.int64)
            sl = np.full(n_lo + n_hi, 999.0, np.float32)
            for j, (n, slot) in enumerate(lo_d):
                xd[j] = n; sl[j] = slot
            for j, (n, slot) in enumerate(hi_d):
                xd[n_lo + j] = n; sl[n_lo + j] = slot
            xlw[g, :, :T_LO * 8] = wrap16(ls)
            xlw[g, :, T_LO * 8:] = wrap16(hs)
            xrw[g] = wrap16(xd)
            # position i -> (t=i//128, p=i%128)
            dstl[g] = sl.reshape(T, P).T
            dstlT[g] = sl
        out.append(dict(xlw=xlw, xrw=xrw, dstl=dstl, dstlT=dstlT,
                        node_ids=node_ids))
    return out, (T, T_LO, T_HI)


def make_core_inputs(core_id, x, w1, w2, gr, cfg):
    NPC, SLOTS, F = cfg.NPC, cfg.SLOTS, cfg.F
    xb = np.zeros((SLOTS, F), np.float32)
    xb[:NPC] = x[core_id * NPC:(core_id + 1) * NPC]
    rowb = lambda v: np.broadcast_to(v.astype(np.float32), (128, F)).copy()
    return {
        "xT_own": np.ascontiguousarray(xb.T),
        "W1l": w1["Wl"], "W1r": w1["Wr"], "W2l": w2["Wl"], "W2r": w2["Wr"],
        "bb1l": rowb(w1["bl"]), "bb1r": rowb(w1["br"]),
        "bb2l": rowb(w2["bl"]), "bb2r": rowb(w2["br"]),
        "inva1": rowb(w1["inva"]), "gbias1": rowb(w1["bias"]),
        "inva2": rowb(w2["inva"]), "gbias2": rowb(w2["bias"]),
        "iotab": np.broadcast_to(np.arange(128, dtype=np.float32), (128, 128)).copy(),
        "ident": np.eye(128, dtype=np.float32),
        "xlw": gr["xlw"], "xrw": gr["xrw"], "dstl": gr["dstl"],
        "dstlT": gr["dstlT"], "node_ids": gr["node_ids"],
        "iotac": np.arange(128, dtype=np.float32).reshape(128, 1),
    }


# ---------------------------------------------------------------- device

def declare_io(nc, cfg):
    CH, P, T, F, SLOTS = cfg.CHUNKS, cfg.P, cfg.T, cfg.F, cfg.SLOTS
    d = {}
    def inp(name, shape, dt=F32):
        d[name] = nc.dram_tensor(name, list(shape), dt, kind="ExternalInput").ap()
    inp("xT_own", (F, SLOTS))
    for n in ("W1l", "W1r", "W2l", "W2r", "bb1l", "bb1r", "bb2l", "bb2r",
              "inva1", "gbias1", "inva2", "gbias2", "iotab", "ident"):
        inp(n, (128, F))
    inp("xlw", (CH, P, T * 8), mybir.dt.int16)
    inp("xrw", (CH, P, T * 8), mybir.dt.int16)
    inp("dstl", (CH, P, T), F32)
    inp("dstlT", (CH, T * P), F32)
    inp("iotac", (128, 1), F32)
    inp("node_ids", (CH, P), I32)
    d["out"] = nc.dram_tensor("out", [SLOTS, F], F32, kind="ExternalOutput").ap()
    return d


def build_program(tc, io, cfg, pos_counts1, pos_counts2):
    cfg._qctr = 0
    nc = tc.nc
    P, F, T, CH = cfg.P, cfg.F, cfg.T, cfg.CHUNKS
    NPC, SLOTS, TD = cfg.NPC, cfg.SLOTS, cfg.TD
    N = cfg.N

    with (
        tc.tile_pool(name="consts", bufs=1) as cpool,
        tc.tile_pool(name="work", bufs=2) as wp,
        tc.tile_pool(name="small", bufs=3) as sp,
        tc.tile_pool(name="psum", bufs=2, space="PSUM") as pp,
        tc.tile_pool(name="dram", bufs=1, space="DRAM") as dp,
    ):
        C = {}
        for n in ("W1l", "W1r", "W2l", "W2r"):
            t = cpool.tile([128, F], TD, tag=n)
            nc.sync.dma_start(t[:], io[n])
            C[n] = t
        for n in ("bb1l", "bb1r", "bb2l", "bb2r", "inva1", "gbias1",
                  "inva2", "gbias2", "iotab"):
            t = cpool.tile([128, F], F32, tag=n)
            nc.sync.dma_start(t[:], io[n])
            C[n] = t
        ident = cpool.tile([128, 128], TD, tag="ident")
        nc.sync.dma_start(ident[:], io["ident"])
        iotac = cpool.tile([128, 1], F32, tag="iotac")
        nc.sync.dma_start(iotac[:], io["iotac"])
        zeros = cpool.tile([128, F], TD, tag="zeros")
        nc.vector.memset(zeros[:], 0.0)

        xl_own = dp.tile([SLOTS, F], TD)
        xr_own = dp.tile([SLOTS + 16, F], TD)
        ag_space = "Shared" if cfg.NC > 4 else "Local"
        h_block = dp.tile([SLOTS + 16, F], TD)
        hl_own = dp.tile([SLOTS, F], TD)
        hr_own = dp.tile([SLOTS + 16, F], TD)

        for tab in (xr_own, hr_own, h_block):
            nc.sync.dma_start(tab[SLOTS:SLOTS + 16, :], zeros[0:16, :])
        if SLOTS > NPC:
            nc.sync.dma_start(h_block[NPC:SLOTS, :], zeros[0:SLOTS - NPC, :])

        def one_pass():
            xl_full = dp.tile([N, F], TD, addr_space=ag_space)
            hl_full = dp.tile([N, F], TD, addr_space=ag_space)
            if getattr(cfg, "skip_tables", False):
                table_phases = False
            else:
                table_phases = True
            for g in range(CH if table_phases else 0):
                xT_sb = sp.tile([128, 128], TD, tag="xT")
                nc.sync.dma_start(xT_sb[:], io["xT_own"][:, g * 128:(g + 1) * 128])
                ps_l = pp.tile([128, F], F32, tag="agg")
                ps_r = pp.tile([128, F], F32, tag="xr", bufs=3)
                nc.tensor.matmul(ps_l[:], lhsT=xT_sb[:], rhs=C["W1l"][:], start=True, stop=True)
                nc.tensor.matmul(ps_r[:], lhsT=xT_sb[:], rhs=C["W1r"][:], start=True, stop=True)
                xl_sb = sp.tile([128, F], TD, tag="xl_sb")
                xr_sb = sp.tile([128, F], TD, tag="xr_sb")
                nc.vector.tensor_tensor(out=xl_sb[:], in0=ps_l[:], in1=C["bb1l"][:], op=mybir.AluOpType.add)
                nc.vector.tensor_tensor(out=xr_sb[:], in0=ps_r[:], in1=C["bb1r"][:], op=mybir.AluOpType.add)
                nc.sync.dma_start(xl_own[g * 128:(g + 1) * 128, :], xl_sb[:])
                nc.sync.dma_start(xr_own[g * 128:(g + 1) * 128, :], xr_sb[:])

            if table_phases:
                if cfg.NC == 1:
                    nc.sync.dma_start(xl_full[:, :], xl_own[0:NPC, :])
                else:
                    nc.gpsimd.collective_compute(
                        "AllGather", mybir.AluOpType.bypass,
                        replica_groups=[list(range(cfg.NC))],
                        ins=[xl_own[0:NPC, :]], outs=[xl_full[:, :]],
                    )

            def edge_layer(tab_full, tab_own, H, pos_counts, inva, gbias, elu, out_to):
                Ch = F // H
                for g in range(CH):
                    TLO, THI = cfg.T_LO, cfg.T_HI
                    HALF = 32768
                    xlw_sb = sp.tile([P, T * 8], mybir.dt.int16, tag="xlw")
                    dstl_sb = sp.tile([P, T], F32, tag="dstl")
                    nid_sb = sp.tile([P, 1], I32, tag="nid")
                    nc.sync.dma_start(xlw_sb[:], io["xlw"][g])
                    nc.sync.dma_start(dstl_sb[:], io["dstl"][g])
                    nc.sync.dma_start(nid_sb[:], io["node_ids"][g].rearrange("(p o) -> p o", o=1))
                    dstb = wp.tile([P, T * F], F32, tag="dstb")
                    nc.sync.dma_start(dstb[:], io["dstlT"][g:g + 1, :].to_broadcast([P, T * P]))
                    urt = sp.tile([P, F], TD, tag="urt")
                    nc.gpsimd.indirect_dma_start(
                        out=urt[:], out_offset=None, in_=tab_own[:, :],
                        in_offset=IndirectOffsetOnAxis(ap=nid_sb[:, 0:1], axis=0))

                    MAXT = 8  # <=1024 idxs per dma_gather (ring capacity)

                    def gathers(out3, in_ap, idx_sb, t0, t1):
                        nq = getattr(cfg, "queues", 1)
                        spk = not getattr(cfg, "sp_false", False)
                        for a in range(t0, t1, MAXT):
                            b = min(a + MAXT, t1)
                            q = cfg._qctr % nq
                            cfg._qctr += 1
                            nc.gpsimd.dma_gather(
                                out_ap=out3[:, a:b, :], in_ap=in_ap,
                                idxs_ap=idx_sb[:, a * 8:b * 8],
                                num_idxs=(b - a) * P, num_idxs_reg=(b - a) * P,
                                elem_size=F, queue_num=q, single_packet=spk)

                    ul = wp.tile([P, T * F], TD, tag="ul")
                    ul3 = ul[:].rearrange("p (t f) -> p t f", f=F)
                    if getattr(cfg, "seq_loads", False):
                        nc.sync.dma_start(ul[:], tab_full[0:T * 128, :].rearrange(
                            "(p t) f -> p (t f)", p=P))
                    else:
                        gathers(ul3, tab_full[0:min(HALF, N), :], xlw_sb, 0, TLO)
                        if THI > 0:
                            gathers(ul3, tab_full[HALF:N, :], xlw_sb, TLO, T)
                    # xr values via one-hot(dst) @ ur_chunk on PE (no gather)
                    ub = wp.tile([P, T * F], TD, tag="ub")
                    ub3 = ub[:].rearrange("p (t f) -> p t f", f=F)
                    for t in range(T):
                        oh_de = sp.tile([P, 128], TD, tag="ohde")
                        nc.vector.tensor_scalar(
                            out=oh_de[:], in0=dstb[:, t * 128:(t + 1) * 128],
                            scalar1=iotac[:, 0:1], scalar2=None,
                            op0=mybir.AluOpType.is_equal)
                        ps_xr = pp.tile([128, F], F32, tag="xr", bufs=3)
                        nc.tensor.matmul(ps_xr[:], lhsT=oh_de[:], rhs=urt[:],
                                         start=True, stop=True)
                        nc.vector.tensor_tensor(out=ub3[:, t, :],
                                                in0=ul3[:, t, :], in1=ps_xr[:],
                                                op=mybir.AluOpType.add)
                    if getattr(cfg, "gather_only", False):
                        gob = sp.tile([P, 1], F32, tag="gob")
                        nc.vector.tensor_reduce(out=gob[:], in_=ub[:],
                                                axis=mybir.AxisListType.X,
                                                op=mybir.AluOpType.add)
                        nc.sync.dma_start(io["out"][g * 128:(g + 1) * 128, 0:1], gob[:])
                        continue

                    lr = wp.tile([P, T * F], TD, tag="lr")
                    if getattr(cfg, "sim_safe", False):
                        nc.vector.tensor_scalar(out=lr[:], in0=ub[:], scalar1=0.2,
                                                scalar2=None, op0=mybir.AluOpType.mult)
                        nc.vector.tensor_tensor(out=lr[:], in0=ub[:], in1=lr[:],
                                                op=mybir.AluOpType.max)
                    else:
                        nc.scalar.activation(out=lr[:], in_=ub[:],
                                             func=mybir.ActivationFunctionType.Prelu,
                                             alpha=0.2)

                    lr3 = lr[:].rearrange("p (t f) -> p t f", f=F)
                    possum = sp.tile([P, T * H], F32, tag="possum")
                    negsum = sp.tile([P, T * H], F32, tag="negsum")
                    pos3 = possum[:].rearrange("p (t h) -> p t h", h=H)
                    neg3 = negsum[:].rearrange("p (t h) -> p t h", h=H)
                    for h in range(H):
                        pc = pos_counts[h]
                        s = h * Ch
                        if pc > 0:
                            nc.vector.tensor_reduce(
                                out=pos3[:, :, h:h + 1], in_=lr3[:, :, s:s + pc],
                                axis=mybir.AxisListType.X, op=mybir.AluOpType.add)
                        else:
                            nc.vector.memset(pos3[:, :, h:h + 1], 0.0)
                        if pc < Ch:
                            nc.vector.tensor_reduce(
                                out=neg3[:, :, h:h + 1], in_=lr3[:, :, s + pc:s + Ch],
                                axis=mybir.AxisListType.X, op=mybir.AluOpType.add)
                        else:
                            nc.vector.memset(neg3[:, :, h:h + 1], 0.0)
                    logit = sp.tile([P, T * H], F32, tag="logit")
                    nc.vector.tensor_tensor(out=logit[:], in0=possum[:], in1=negsum[:],
                                            op=mybir.AluOpType.subtract)

                    aug = wp.tile([P, T * (F + H)], TD, tag="aug")
                    aug3 = aug[:].rearrange("p (t c) -> p t c", c=F + H)
                    nc.scalar.activation(out=aug3[:, :, F:F + H], in_=logit[:],
                                         func=mybir.ActivationFunctionType.Exp)
                    ub4 = ub[:].rearrange("p (t h c) -> p t h c", h=H, c=Ch)
                    aug4 = aug3[:, :, 0:F].rearrange("p t (h c) -> p t h c", h=H)
                    wb = aug3[:, :, F:F + H].to_broadcast([P, T, H, Ch])
                    nc.vector.tensor_tensor(out=aug4, in0=ub4, in1=wb,
                                            op=mybir.AluOpType.mult)

                    ps = pp.tile([128, F + H], F32, tag="agg")
                    for t in range(T):
                        oh = sp.tile([P, 128], TD, tag="oh")
                        nc.vector.tensor_scalar(
                            out=oh[:], in0=C["iotab"][:], scalar1=dstl_sb[:, t:t + 1],
                            scalar2=None, op0=mybir.AluOpType.is_equal)
                        nc.tensor.matmul(ps[:], lhsT=oh[:],
                                         rhs=aug3[:, t, :],
                                         start=(t == 0), stop=(t == T - 1))

                    den = sp.tile([P, H], F32, tag="den")
                    nc.vector.tensor_scalar(out=den[:], in0=ps[:, F:F + H],
                                            scalar1=1e-30, scalar2=None,
                                            op0=mybir.AluOpType.add)
                    rec = sp.tile([P, H], F32, tag="rec")
                    nc.vector.reciprocal(rec[:], den[:])
                    o1 = sp.tile([P, F], F32, tag="o1")
                    if H > 1:
                        nc.vector.tensor_tensor(
                            out=o1[:].rearrange("p (h c) -> p h c", h=H),
                            in0=ps[:, 0:F].rearrange("p (h c) -> p h c", h=H),
                            in1=rec[:].to_broadcast([P, H, Ch]),
                            op=mybir.AluOpType.mult)
                    else:
                        nc.vector.tensor_scalar(out=o1[:], in0=ps[:, 0:F],
                                                scalar1=rec[:, 0:1], scalar2=None,
                                                op0=mybir.AluOpType.mult)
                    if TD != F32:
                        urf = sp.tile([P, F], F32, tag="urf")
                        nc.vector.tensor_copy(out=urf[:], in_=urt[:])
                    else:
                        urf = urt
                    nc.vector.tensor_tensor(out=o1[:], in0=o1[:], in1=urf[:],
                                            op=mybir.AluOpType.subtract)
                    nc.vector.tensor_tensor(out=o1[:], in0=o1[:], in1=inva[:],
                                            op=mybir.AluOpType.mult)
                    nc.vector.tensor_tensor(out=o1[:], in0=o1[:], in1=gbias[:],
                                            op=mybir.AluOpType.add)
                    if elu:
                        m0 = sp.tile([P, F], F32, tag="m0")
                        nc.vector.tensor_scalar(out=m0[:], in0=o1[:], scalar1=0.0,
                                                scalar2=None, op0=mybir.AluOpType.min)
                        e0 = sp.tile([P, F], F32, tag="e0")
                        nc.scalar.activation(out=e0[:], in_=m0[:],
                                             func=mybir.ActivationFunctionType.Exp)
                        nc.vector.tensor_scalar(out=o1[:], in0=o1[:], scalar1=0.0,
                                                scalar2=None, op0=mybir.AluOpType.max)
                        nc.vector.tensor_tensor(out=o1[:], in0=o1[:], in1=e0[:],
                                                op=mybir.AluOpType.add)
                        nc.vector.tensor_scalar(out=o1[:], in0=o1[:], scalar1=1.0,
                                                scalar2=None, op0=mybir.AluOpType.subtract)
                    if out_to == "h_block":
                        if TD != F32:
                            hcast = sp.tile([P, F], TD, tag="hcast")
                            nc.vector.tensor_copy(out=hcast[:], in_=o1[:])
                            src_tile = hcast
                        else:
                            src_tile = o1
                        nc.gpsimd.indirect_dma_start(
                            out=h_block[:, :],
                            out_offset=IndirectOffsetOnAxis(ap=nid_sb[:, 0:1], axis=0),
                            in_=src_tile[:], in_offset=None)
                    else:
                        nc.sync.dma_start(io["out"][g * 128:(g + 1) * 128, :], o1[:])

            edge_layer(xl_full, xr_own, cfg.H1, pos_counts1,
                       C["inva1"], C["gbias1"], elu=True, out_to="h_block")

            for g in range(CH if table_phases else 0):
                h_sb = sp.tile([128, F], TD, tag="h_sb")
                nc.sync.dma_start(h_sb[:], h_block[g * 128:(g + 1) * 128, :])
                ps_t = pp.tile([128, 128], F32, tag="xr", bufs=3)
                nc.tensor.transpose(out=ps_t[:], in_=h_sb[:], identity=ident[:])
                hT_sb = sp.tile([128, 128], TD, tag="hT")
                nc.vector.tensor_copy(out=hT_sb[:], in_=ps_t[:])
                ps_l = pp.tile([128, F], F32, tag="agg")
                ps_r = pp.tile([128, F], F32, tag="xr", bufs=3)
                nc.tensor.matmul(ps_l[:], lhsT=hT_sb[:], rhs=C["W2l"][:], start=True, stop=True)
                nc.tensor.matmul(ps_r[:], lhsT=hT_sb[:], rhs=C["W2r"][:], start=True, stop=True)
                hl_sb = sp.tile([128, F], TD, tag="xl_sb")
                hr_sb = sp.tile([128, F], TD, tag="xr_sb")
                nc.vector.tensor_tensor(out=hl_sb[:], in0=ps_l[:], in1=C["bb2l"][:], op=mybir.AluOpType.add)
                nc.vector.tensor_tensor(out=hr_sb[:], in0=ps_r[:], in1=C["bb2r"][:], op=mybir.AluOpType.add)
                nc.sync.dma_start(hl_own[g * 128:(g + 1) * 128, :], hl_sb[:])
                nc.sync.dma_start(hr_own[g * 128:(g + 1) * 128, :], hr_sb[:])

            if table_phases:
                if cfg.NC == 1:
                    nc.sync.dma_start(hl_full[:, :], hl_own[0:NPC, :])
                else:
                    nc.gpsimd.collective_compute(
                        "AllGather", mybir.AluOpType.bypass,
                        replica_groups=[list(range(cfg.NC))],
                        ins=[hl_own[0:NPC, :]], outs=[hl_full[:, :]],
                    )

            edge_layer(hl_full, hr_own, 1, pos_counts2,
                       C["inva2"], C["gbias2"], elu=False, out_to="out")

        for _rep in range(getattr(cfg, "repeats", 1)):
            one_pass()


# ---------------------------------------------------------------- runner

_LAST = {}


def kernel(**inputs) -> np.ndarray:
    x = np.asarray(inputs["x"], np.float32)
    ei = np.asarray(inputs["edge_index"])
    w1 = prep_weights(np.asarray(inputs["att1"], np.float32),
                      np.asarray(inputs["W1l"], np.float32),
                      np.asarray(inputs["b1l"], np.float32),
                      np.asarray(inputs["W1r"], np.float32),
                      np.asarray(inputs["b1r"], np.float32),
                      np.asarray(inputs["bias1"], np.float32))
    w2 = prep_weights(np.asarray(inputs["att2"], np.float32),
                      np.asarray(inputs["W2l"], np.float32)[w1["perm"], :],
                      np.asarray(inputs["b2l"], np.float32),
                      np.asarray(inputs["W2r"], np.float32)[w1["perm"], :],
                      np.asarray(inputs["b2r"], np.float32),
                      np.asarray(inputs["bias2"], np.float32))
    cfg = Cfg(N_NODES, N_CORES, FEAT, HEADS1, T=None)
    cfg.queues = 4
    grs, (T, T_LO, T_HI) = prep_graph(ei, cfg)
    cfg.T, cfg.T_LO, cfg.T_HI = T, T_LO, T_HI

    in_maps = [make_core_inputs(c, x, w1, w2, grs[c], cfg) for c in range(N_CORES)]

    nc = bacc.Bacc("TRN2", target_bir_lowering=False, debug=False,
                   num_devices=N_CORES,
                   num_swdge_queues=getattr(cfg, "queues", 1))
    io = declare_io(nc, cfg)
    with tile.TileContext(nc) as tc:
        build_program(tc, io, cfg, w1["pos_counts"], w2["pos_counts"])
    nc.compile()

    res = bass_utils.run_bass_kernel_spmd(nc, in_maps, core_ids=list(range(N_CORES)))
    _LAST["results"] = res
    _LAST["nc"] = nc
    _LAST["in_maps"] = in_maps
    _LAST["cfg"] = cfg

    out = np.zeros((cfg.N, cfg.F), np.float32)
    for c in range(N_CORES):
        oc = np.asarray(res.results[c]["out"])
        ni = grs[c]["node_ids"].ravel()
        valid = ni < cfg.NPC
        out[c * cfg.NPC + ni[valid]] = oc.reshape(cfg.SLOTS, cfg.F)[valid]
    final = np.empty_like(out)
    final[:, w2["perm"]] = out
    return final



# revision 5
# speedup vs baseline: 1.0278x; 1.0278x over previous
"""GATv2 encoder (2-layer, PyG GATv2Conv semantics) on 8 TRN2 NeuronCores — v2.

Sharding: dst-node blocks, one slot-permutation per core so chunk rows are
contiguous (no indirect DMA); edges live with their dst core; one AllGather
of the folded source-side node table per layer.

v2 changes vs v1: bf16 tables/gathers/matmuls, slot permutation (kills
nid load + urt indirect + output scatter), single fused one-hot builds
(2 DVE ops per chunk instead of 2T), ul+ur summed in PSUM via paired
matmuls (identity trick), Prelu straight from PSUM, sign-vector logits
(one multiply + one 4D reduce), u reconstructed from lrelu via
max(lr, 5*lr) instead of keeping ub in SBUF.

Math identical to v1: |att| folded into Wl/Wr columns so
logits = sum_c sign_c * lrelu(u~_c), u~ = ul~[src] + ur~[dst];
sum_e alpha*(ul~+ur~) = sum_e alpha*ul~ + ur~, recovered via 1/|att|.
"""
import numpy as np
import ml_dtypes

try:
    import concourse  # noqa: F401
except ImportError:  # pragma: no cover
    import sys
    sys.path.insert(0, "/opt/trn_rl_repo")

from concourse import bass, bacc, mybir, tile
from concourse import bass_utils

F32 = mybir.dt.float32
BF16 = mybir.dt.bfloat16
I16 = mybir.dt.int16
NPBF = ml_dtypes.bfloat16

N_NODES = 50000
N_CORES = 8
FEAT = 128
HEADS1 = 4


class Cfg:
    def __init__(self, n_nodes, n_cores, feat, heads1):
        self.N = n_nodes
        self.NC = n_cores
        self.NPC = n_nodes // n_cores
        self.P = 128
        self.CHUNKS = (self.NPC + 127) // 128
        self.SLOTS = self.CHUNKS * 128
        self.TOT = self.SLOTS * n_cores      # rows in the gathered table
        self.F = feat
        self.H1 = heads1
        self.T = None
        self.TD = BF16
        self.queues = 4
        self.repeats = 1


# ---------------------------------------------------------------- host prep

def prep_weights(att, Wl, bl, Wr, br, bias):
    a = att.reshape(-1).astype(np.float64)
    absa = np.maximum(np.abs(a), 1e-12)
    sign = np.where(a >= 0, 1.0, -1.0)
    return dict(
        Wl=(Wl * absa[None, :]).astype(np.float32),
        bl=(bl * absa).astype(np.float32),
        Wr=(Wr * absa[None, :]).astype(np.float32),
        br=(br * absa).astype(np.float32),
        inva=(1.0 / absa).astype(np.float32),
        sign=sign.astype(np.float32),
        bias=bias.astype(np.float32),
    )


def prep_graph(edge_index, cfg):
    """Slot permutation + per-chunk edge layout for dma_gather (int16 idx).

    Nodes of each core are bin-packed into CHUNKS bins of <=128 slots,
    balancing edges per bin; slot = (bin, lane). Edges are placed on their
    dst core/chunk, split into [src_newid < 32768 | >= 32768] sections,
    each padded to global tile counts T_LO / T_HI. newid = core*SLOTS+slot.
    """
    import heapq
    N, NPC, P, CHUNKS, SLOTS, NC = (cfg.N, cfg.NPC, cfg.P, cfg.CHUNKS,
                                    cfg.SLOTS, cfg.NC)
    HALF = 32768
    src = np.asarray(edge_index[0], dtype=np.int64)
    dst = np.asarray(edge_index[1], dtype=np.int64)
    loops = np.arange(N, dtype=np.int64)
    src = np.concatenate([src, loops])
    dst = np.concatenate([dst, loops])

    # pass 1: slot assignment per core
    newid = np.full(N, -1, dtype=np.int64)
    node_ids_all = []
    per_core_edges = []
    for c in range(NC):
        lo = c * NPC
        m = (dst >= lo) & (dst < lo + NPC)
        s_c = src[m]
        d_c = dst[m] - lo
        per_core_edges.append((s_c, d_c))
        deg = np.bincount(d_c, minlength=NPC)
        order = np.argsort(-deg, kind="stable")
        heap = [(0, g) for g in range(CHUNKS)]
        heapq.heapify(heap)
        bin_cnt = [0] * CHUNKS
        bin_sum = [0] * CHUNKS
        node_ids = np.full((CHUNKS, P), -1, dtype=np.int64)
        for n in order:
            while True:
                sm, g = heapq.heappop(heap)
                if bin_cnt[g] < P:
                    break
            node_ids[g, bin_cnt[g]] = n
            newid[lo + n] = c * SLOTS + g * P + bin_cnt[g]
            bin_cnt[g] += 1
            bin_sum[g] = sm + int(deg[n])
            if bin_cnt[g] < P:
                heapq.heappush(heap, (bin_sum[g], g))
        node_ids_all.append(node_ids)

    # pass 2: per-chunk edge sections with src newids
    cores_chunk_edges = []
    maxTlo = maxThi = 0
    for c in range(NC):
        s_c, d_c = per_core_edges[c]
        sid = newid[s_c]                      # src new global id
        dslot = newid[c * NPC + d_c] - c * SLOTS  # local slot in [0, SLOTS)
        g_of = dslot // P
        chunk_edges = []
        for g in range(CHUNKS):
            m = g_of == g
            sg = sid[m]
            tg = dslot[m] - g * P             # lane 0..127
            lo_m = sg < HALF
            lo_s, lo_t = sg[lo_m], tg[lo_m]
            hi_s, hi_t = sg[~lo_m] - HALF, tg[~lo_m]
            maxTlo = max(maxTlo, (len(lo_s) + P - 1) // P)
            maxThi = max(maxThi, (len(hi_s) + P - 1) // P)
            chunk_edges.append((lo_s, lo_t, hi_s, hi_t))
        cores_chunk_edges.append(chunk_edges)

    T_LO = max(maxTlo, 1)
    T_HI = maxThi if cfg.TOT > HALF else 0
    if cfg.TOT > HALF:
        T_HI = max(T_HI, 1)
    T = T_LO + T_HI

    def wrap16(ids):
        a = np.asarray(ids, dtype=np.int16).reshape(-1, 16).T
        return np.tile(a, (8, 1))

    out = []
    for c in range(NC):
        chunk_edges = cores_chunk_edges[c]
        xlw = np.zeros((CHUNKS, P, T * 8), dtype=np.int16)
        dstl = np.full((CHUNKS, P, T), 999.0, dtype=NPBF)
        dstlT = np.full((CHUNKS, T * P), 999.0, dtype=NPBF)
        for g in range(CHUNKS):
            lo_s, lo_t, hi_s, hi_t = chunk_edges[g]
            n_lo, n_hi = T_LO * P, T_HI * P
            ls = np.zeros(n_lo, np.int64); ls[:len(lo_s)] = lo_s
            sl = np.full(n_lo + n_hi, 999.0, np.float32)
            sl[:len(lo_t)] = lo_t
            xlw[g, :, :T_LO * 8] = wrap16(ls)
            if T_HI > 0:
                hs = np.zeros(n_hi, np.int64); hs[:len(hi_s)] = hi_s
                sl[n_lo:n_lo + len(hi_t)] = hi_t
                xlw[g, :, T_LO * 8:] = wrap16(hs)
            # edge i -> (t = i//128, lane = i%128)
            dstl[g] = sl.reshape(T, P).T.astype(NPBF)
            dstlT[g] = sl.astype(NPBF)
        out.append(dict(xlw=xlw, dstl=dstl, dstlT=dstlT,
                        node_ids=node_ids_all[c]))
    return out, (T, T_LO, T_HI)


def make_core_inputs(core_id, x, w1, w2, gr, cfg):
    SLOTS, F, P = cfg.SLOTS, cfg.F, cfg.P
    nid = gr["node_ids"].ravel()
    xb = np.zeros((SLOTS, F), np.float32)
    valid = nid >= 0
    xb[valid] = x[core_id * cfg.NPC + nid[valid]]
    rowb = lambda v: np.broadcast_to(v.astype(np.float32), (P, F)).copy()
    rowb16 = lambda v: np.broadcast_to(v.astype(NPBF), (P, F)).copy()
    return {
        "xT_own": np.ascontiguousarray(xb.T).astype(NPBF),
        "W1l": w1["Wl"].astype(NPBF), "W1r": w1["Wr"].astype(NPBF),
        "W2l": w2["Wl"].astype(NPBF), "W2r": w2["Wr"].astype(NPBF),
        "bb1l": rowb(w1["bl"]), "bb1r": rowb(w1["br"]),
        "bb2l": rowb(w2["bl"]), "bb2r": rowb(w2["br"]),
        "inva1": rowb(w1["inva"]), "gbias1": rowb(w1["bias"]),
        "inva2": rowb(w2["inva"]), "gbias2": rowb(w2["bias"]),
        "sgn1": rowb16(w1["sign"]), "sgn2": rowb16(w2["sign"]),
        "identb": np.eye(P, dtype=NPBF),
        "iotac": np.arange(P, dtype=np.float32).reshape(P, 1),
        "iotab": np.broadcast_to(np.arange(P, dtype=NPBF), (P, P)).copy(),
        "xlw": gr["xlw"], "dstl": gr["dstl"], "dstlT": gr["dstlT"],
    }


# ---------------------------------------------------------------- device

def declare_io(nc, cfg):
    CH, P, T, F, SLOTS = cfg.CHUNKS, cfg.P, cfg.T, cfg.F, cfg.SLOTS
    TD = cfg.TD
    d = {}
    def inp(name, shape, dt):
        d[name] = nc.dram_tensor(name, list(shape), dt, kind="ExternalInput").ap()
    inp("xT_own", (F, SLOTS), TD)
    for n in ("W1l", "W1r", "W2l", "W2r", "sgn1", "sgn2", "iotab"):
        inp(n, (P, F), TD)
    for n in ("bb1l", "bb1r", "bb2l", "bb2r",
              "inva1", "gbias1", "inva2", "gbias2"):
        inp(n, (P, F), F32)
    inp("identb", (P, P), TD)
    inp("iotac", (P, 1), F32)
    inp("xlw", (CH, P, T * 8), I16)
    inp("dstl", (CH, P, T), TD)
    inp("dstlT", (CH, T * P), TD)
    d["out"] = nc.dram_tensor("out", [SLOTS, F], F32, kind="ExternalOutput").ap()
    return d


def build_program(tc, io, cfg):
    nc = tc.nc
    P, F, T, CH = cfg.P, cfg.F, cfg.T, cfg.CHUNKS
    SLOTS, TD, TOT = cfg.SLOTS, cfg.TD, cfg.TOT
    TLO, THI = cfg.T_LO, cfg.T_HI
    HALF = 32768
    H1 = cfg.H1
    MAXT = 8
    qctr = [0]

    with (
        tc.tile_pool(name="consts", bufs=1) as cpool,
        tc.tile_pool(name="work", bufs=3) as wp,
        tc.tile_pool(name="small", bufs=3) as sp,
        tc.tile_pool(name="psum", bufs=2, space="PSUM") as pp,
        tc.tile_pool(name="dram", bufs=1, space="DRAM") as dp,
    ):
        C = {}
        for n in ("W1l", "W1r", "W2l", "W2r", "sgn1", "sgn2", "iotab"):
            t = cpool.tile([P, F], TD, tag=n)
            nc.sync.dma_start(t[:], io[n])
            C[n] = t
        for n in ("bb1l", "bb1r", "bb2l", "bb2r",
                  "inva1", "gbias1", "inva2", "gbias2"):
            t = cpool.tile([P, F], F32, tag=n)
            nc.sync.dma_start(t[:], io[n])
            C[n] = t
        identb = cpool.tile([P, P], TD, tag="identb")
        nc.sync.dma_start(identb[:], io["identb"])
        iotac = cpool.tile([P, 1], F32, tag="iotac")
        nc.sync.dma_start(iotac[:], io["iotac"])

        xl_own = dp.tile([SLOTS, F], TD)
        xr_own = dp.tile([SLOTS, F], TD)
        h_block = dp.tile([SLOTS, F], TD)
        hl_own = dp.tile([SLOTS, F], TD)
        hr_own = dp.tile([SLOTS, F], TD)
        ag_space = "Shared" if cfg.NC > 1 else "Local"

        def table_phase(src_rows, Wl, Wr, bbl, bbr, dst_l, dst_r, transpose):
            for g in range(CH):
                xT_sb = sp.tile([P, P], TD, tag="xT")
                if transpose:
                    h_sb = sp.tile([P, P], TD, tag="h_sb")
                    nc.sync.dma_start(h_sb[:], src_rows[g * P:(g + 1) * P, :])
                    ps_t = pp.tile([P, P], TD, tag="pst")
                    nc.tensor.transpose(out=ps_t[:], in_=h_sb[:],
                                        identity=identb[:])
                    nc.vector.tensor_copy(out=xT_sb[:], in_=ps_t[:])
                else:
                    nc.sync.dma_start(xT_sb[:], src_rows[:, g * P:(g + 1) * P])
                ps_l = pp.tile([P, F], F32, tag="agg")
                ps_r = pp.tile([P, F], F32, tag="psg")
                nc.tensor.matmul(ps_l[:], lhsT=xT_sb[:], rhs=Wl[:],
                                 start=True, stop=True)
                nc.tensor.matmul(ps_r[:], lhsT=xT_sb[:], rhs=Wr[:],
                                 start=True, stop=True)
                xl_sb = sp.tile([P, F], TD, tag="xl_sb")
                xr_sb = sp.tile([P, F], TD, tag="xr_sb")
                nc.vector.tensor_tensor(out=xl_sb[:], in0=ps_l[:], in1=bbl[:],
                                        op=mybir.AluOpType.add)
                nc.vector.tensor_tensor(out=xr_sb[:], in0=ps_r[:], in1=bbr[:],
                                        op=mybir.AluOpType.add)
                nc.sync.dma_start(dst_l[g * P:(g + 1) * P, :], xl_sb[:])
                nc.sync.dma_start(dst_r[g * P:(g + 1) * P, :], xr_sb[:])

        def all_gather(own, full):
            if cfg.NC == 1:
                nc.sync.dma_start(full[:, :], own[0:SLOTS, :])
            else:
                nc.gpsimd.collective_compute(
                    "AllGather", mybir.AluOpType.bypass,
                    replica_groups=[list(range(cfg.NC))],
                    ins=[own[0:SLOTS, :]], outs=[full[:, :]],
                )

        def edge_layer(tab_full, tab_own, H, sgn, inva, gbias, elu, out_to):
            Ch = F // H
            NG = (T + 3) // 4                      # 4-tile PSUM groups
            for g in range(CH):
                xlw_sb = sp.tile([P, T * 8], I16, tag="xlw")
                dstl_sb = sp.tile([P, T], TD, tag="dstl")
                nc.sync.dma_start(xlw_sb[:], io["xlw"][g])
                nc.sync.dma_start(dstl_sb[:], io["dstl"][g])
                dstb = wp.tile([P, T * P], TD, tag="dstb")
                nc.sync.dma_start(
                    dstb[:], io["dstlT"][g:g + 1, :].to_broadcast([P, T * P]))
                urt = sp.tile([P, F], TD, tag="urt")
                nc.sync.dma_start(urt[:], tab_own[g * P:(g + 1) * P, :])

                ul = wp.tile([P, T * F], TD, tag="ul")
                ul3 = ul[:].rearrange("p (t f) -> p t f", f=F)
                for a in range(0, TLO, MAXT):
                    b = min(a + MAXT, TLO)
                    nc.gpsimd.dma_gather(
                        out_ap=ul3[:, a:b, :], in_ap=tab_full[0:min(HALF, TOT), :],
                        idxs_ap=xlw_sb[:, a * 8:b * 8],
                        num_idxs=(b - a) * P, num_idxs_reg=(b - a) * P,
                        elem_size=F, queue_num=qctr[0] % cfg.queues,
                        single_packet=True)
                    qctr[0] += 1
                for a in range(TLO, T, MAXT):
                    b = min(a + MAXT, T)
                    nc.gpsimd.dma_gather(
                        out_ap=ul3[:, a:b, :], in_ap=tab_full[HALF:TOT, :],
                        idxs_ap=xlw_sb[:, a * 8:b * 8],
                        num_idxs=(b - a) * P, num_idxs_reg=(b - a) * P,
                        elem_size=F, queue_num=qctr[0] % cfg.queues,
                        single_packet=True)
                    qctr[0] += 1

                # one-hot builds: 2 DVE ops for the whole chunk
                oh_de = wp.tile([P, T * P], TD, tag="oh_de")
                nc.vector.tensor_scalar(
                    out=oh_de[:], in0=dstb[:], scalar1=iotac[:, 0:1],
                    scalar2=None, op0=mybir.AluOpType.is_equal)
                oh_ag = wp.tile([P, T * P], TD, tag="oh_ag")
                nc.vector.tensor_tensor(
                    out=oh_ag[:].rearrange("p (t f) -> p t f", f=P),
                    in0=C["iotab"][:].rearrange("p (o f) -> p o f", o=1)
                        .to_broadcast([P, T, P]),
                    in1=dstl_sb[:].rearrange("p (t o) -> p t o", o=1)
                        .to_broadcast([P, T, P]),
                    op=mybir.AluOpType.is_equal)
                oh_de3 = oh_de[:].rearrange("p (t f) -> p t f", f=P)
                oh_ag3 = oh_ag[:].rearrange("p (t f) -> p t f", f=P)

                # u~ = ul[src] + ur[dst] summed in PSUM; lrelu from PSUM
                lr = wp.tile([P, T * F], TD, tag="lr")
                for grp in range(NG):
                    t0, t1 = grp * 4, min(grp * 4 + 4, T)
                    ncols = (t1 - t0) * F
                    psg = pp.tile([P, 4 * F], F32, tag="psg")
                    for t in range(t0, t1):
                        c0 = (t - t0) * F
                        nc.tensor.matmul(psg[:, c0:c0 + F], lhsT=oh_de3[:, t, :],
                                         rhs=urt[:], start=True, stop=False)
                        nc.tensor.matmul(psg[:, c0:c0 + F], lhsT=identb[:],
                                         rhs=ul3[:, t, :], start=False, stop=True)
                    if getattr(cfg, "sim_safe", False):
                        t02 = sp.tile([P, 4 * F], F32, tag="t02")
                        nc.vector.tensor_scalar(
                            out=t02[:, 0:ncols], in0=psg[:, 0:ncols],
                            scalar1=0.2, scalar2=None,
                            op0=mybir.AluOpType.mult)
                        nc.vector.tensor_tensor(
                            out=lr[:, t0 * F:t0 * F + ncols],
                            in0=psg[:, 0:ncols], in1=t02[:, 0:ncols],
                            op=mybir.AluOpType.max)
                    else:
                        nc.scalar.activation(
                            out=lr[:, t0 * F:t0 * F + ncols], in_=psg[:, 0:ncols],
                            func=mybir.ActivationFunctionType.Prelu, alpha=0.2)

                # logits = reduce(sign * lr) per (tile, head)
                sgt = wp.tile([P, T * F], TD, tag="sgt")
                nc.vector.tensor_tensor(
                    out=sgt[:].rearrange("p (t f) -> p t f", f=F),
                    in0=lr[:].rearrange("p (t f) -> p t f", f=F),
                    in1=sgn[:].rearrange("p (o f) -> p o f", o=1)
                        .to_broadcast([P, T, F]),
                    op=mybir.AluOpType.mult)
                logit = sp.tile([P, T * H], F32, tag="logit")
                nc.vector.tensor_reduce(
                    out=logit[:].rearrange("p (t h o) -> p t h o", h=H, o=1),
                    in_=sgt[:].rearrange("p (t h c) -> p t h c", h=H, c=Ch),
                    axis=mybir.AxisListType.X, op=mybir.AluOpType.add)

                aug = wp.tile([P, T * (F + H)], TD, tag="aug")
                aug3 = aug[:].rearrange("p (t c) -> p t c", c=F + H)
                nc.scalar.activation(out=aug3[:, :, F:F + H], in_=logit[:],
                                     func=mybir.ActivationFunctionType.Exp)
                # u~ reconstructed from lrelu: u = max(lr, 5*lr)
                ubr = wp.tile([P, T * F], TD, tag="ubr")
                nc.vector.tensor_scalar(out=ubr[:], in0=lr[:], scalar1=5.0,
                                        scalar2=None, op0=mybir.AluOpType.mult)
                nc.vector.tensor_tensor(out=ubr[:], in0=ubr[:], in1=lr[:],
                                        op=mybir.AluOpType.min)
                ub4 = ubr[:].rearrange("p (t h c) -> p t h c", h=H, c=Ch)
                aug4 = aug3[:, :, 0:F].rearrange("p t (h c) -> p t h c", h=H)
                wb = aug3[:, :, F:F + H].to_broadcast([P, T, H, Ch])
                nc.vector.tensor_tensor(out=aug4, in0=ub4, in1=wb,
                                        op=mybir.AluOpType.mult)

                ps = pp.tile([P, F + H], F32, tag="agg")
                for t in range(T):
                    nc.tensor.matmul(ps[:], lhsT=oh_ag3[:, t, :],
                                     rhs=aug3[:, t, :],
                                     start=(t == 0), stop=(t == T - 1))

                den = sp.tile([P, H], F32, tag="den")
                nc.vector.tensor_scalar(out=den[:], in0=ps[:, F:F + H],
                                        scalar1=1e-30, scalar2=None,
                                        op0=mybir.AluOpType.add)
                rec = sp.tile([P, H], F32, tag="rec")
                nc.vector.reciprocal(rec[:], den[:])
                o1 = sp.tile([P, F], F32, tag="o1")
                if H > 1:
                    nc.vector.tensor_tensor(
                        out=o1[:].rearrange("p (h c) -> p h c", h=H),
                        in0=ps[:, 0:F].rearrange("p (h c) -> p h c", h=H),
                        in1=rec[:].rearrange("p (h o) -> p h o", o=1)
                            .to_broadcast([P, H, Ch]),
                        op=mybir.AluOpType.mult)
                else:
                    nc.vector.tensor_scalar(out=o1[:], in0=ps[:, 0:F],
                                            scalar1=rec[:, 0:1], scalar2=None,
                                            op0=mybir.AluOpType.mult)
                urf = sp.tile([P, F], F32, tag="urf")
                nc.vector.tensor_copy(out=urf[:], in_=urt[:])
                nc.vector.tensor_tensor(out=o1[:], in0=o1[:], in1=urf[:],
                                        op=mybir.AluOpType.subtract)
                nc.vector.tensor_tensor(out=o1[:], in0=o1[:], in1=inva[:],
                                        op=mybir.AluOpType.mult)
                nc.vector.tensor_tensor(out=o1[:], in0=o1[:], in1=gbias[:],
                                        op=mybir.AluOpType.add)
                if elu:
                    m0 = sp.tile([P, F], F32, tag="m0")
                    nc.vector.tensor_scalar(out=m0[:], in0=o1[:], scalar1=0.0,
                                            scalar2=None, op0=mybir.AluOpType.min)
                    e0 = sp.tile([P, F], F32, tag="e0")
                    nc.scalar.activation(out=e0[:], in_=m0[:],
                                         func=mybir.ActivationFunctionType.Exp)
                    nc.vector.tensor_scalar(out=o1[:], in0=o1[:], scalar1=0.0,
                                            scalar2=None, op0=mybir.AluOpType.max)
                    nc.vector.tensor_tensor(out=o1[:], in0=o1[:], in1=e0[:],
                                            op=mybir.AluOpType.add)
                    nc.vector.tensor_scalar(out=o1[:], in0=o1[:], scalar1=1.0,
                                            scalar2=None,
                                            op0=mybir.AluOpType.subtract)
                if out_to is h_block:
                    hcast = sp.tile([P, F], TD, tag="hcast")
                    nc.vector.tensor_copy(out=hcast[:], in_=o1[:])
                    nc.sync.dma_start(out_to[g * P:(g + 1) * P, :], hcast[:])
                else:
                    nc.sync.dma_start(out_to[g * P:(g + 1) * P, :], o1[:])

        for _rep in range(cfg.repeats):
            xl_full = dp.tile([TOT, F], TD, addr_space=ag_space)
            hl_full = dp.tile([TOT, F], TD, addr_space=ag_space)
            table_phase(io["xT_own"], C["W1l"], C["W1r"], C["bb1l"], C["bb1r"],
                        xl_own, xr_own, transpose=False)
            all_gather(xl_own, xl_full)
            edge_layer(xl_full, xr_own, H1, C["sgn1"], C["inva1"], C["gbias1"],
                       elu=True, out_to=h_block)
            table_phase(h_block, C["W2l"], C["W2r"], C["bb2l"], C["bb2r"],
                        hl_own, hr_own, transpose=True)
            all_gather(hl_own, hl_full)
            edge_layer(hl_full, hr_own, 1, C["sgn2"], C["inva2"], C["gbias2"],
                       elu=False, out_to=io["out"])


# ---------------------------------------------------------------- runner

_LAST = {}


def _build(inputs, cfg):
    x = np.asarray(inputs["x"], np.float32)
    ei = np.asarray(inputs["edge_index"])
    w1 = prep_weights(np.asarray(inputs["att1"], np.float32),
                      np.asarray(inputs["W1l"], np.float32),
                      np.asarray(inputs["b1l"], np.float32),
                      np.asarray(inputs["W1r"], np.float32),
                      np.asarray(inputs["b1r"], np.float32),
                      np.asarray(inputs["bias1"], np.float32))
    w2 = prep_weights(np.asarray(inputs["att2"], np.float32),
                      np.asarray(inputs["W2l"], np.float32),
                      np.asarray(inputs["b2l"], np.float32),
                      np.asarray(inputs["W2r"], np.float32),
                      np.asarray(inputs["b2r"], np.float32),
                      np.asarray(inputs["bias2"], np.float32))
    grs, (T, T_LO, T_HI) = prep_graph(ei, cfg)
    cfg.T, cfg.T_LO, cfg.T_HI = T, T_LO, T_HI
    in_maps = [make_core_inputs(c, x, w1, w2, grs[c], cfg)
               for c in range(cfg.NC)]
    nc = bacc.Bacc("TRN2", target_bir_lowering=False, debug=False,
                   num_devices=cfg.NC, num_swdge_queues=cfg.queues)
    io = declare_io(nc, cfg)
    with tile.TileContext(nc) as tc:
        build_program(tc, io, cfg)
    nc.compile()
    return nc, in_maps, grs


def kernel(**inputs) -> np.ndarray:
    cfg = Cfg(N_NODES, N_CORES, FEAT, HEADS1)
    nc, in_maps, grs = _build(inputs, cfg)
    res = bass_utils.run_bass_kernel_spmd(nc, in_maps,
                                          core_ids=list(range(cfg.NC)))
    _LAST.update(results=res, nc=nc, in_maps=in_maps, cfg=cfg, grs=grs)

    out = np.zeros((cfg.N, cfg.F), np.float32)
    for c in range(cfg.NC):
        oc = np.asarray(res.results[c]["out"]).reshape(cfg.SLOTS, cfg.F)
        nid = grs[c]["node_ids"].ravel()
        valid = nid >= 0
        out[c * cfg.NPC + nid[valid]] = oc[valid]
    return out


# revision 7
# speedup vs baseline: 1.0796x; 1.0504x over previous
"""GATv2 encoder (2-layer, PyG GATv2Conv semantics) on 8 TRN2 NeuronCores — v2.

Sharding: dst-node blocks, one slot-permutation per core so chunk rows are
contiguous (no indirect DMA); edges live with their dst core; one AllGather
of the folded source-side node table per layer.

v2 changes vs v1: bf16 tables/gathers/matmuls, slot permutation (kills
nid load + urt indirect + output scatter), single fused one-hot builds
(2 DVE ops per chunk instead of 2T), ul+ur summed in PSUM via paired
matmuls (identity trick), Prelu straight from PSUM, sign-vector logits
(one multiply + one 4D reduce), u reconstructed from lrelu via
max(lr, 5*lr) instead of keeping ub in SBUF.

Math identical to v1: |att| folded into Wl/Wr columns so
logits = sum_c sign_c * lrelu(u~_c), u~ = ul~[src] + ur~[dst];
sum_e alpha*(ul~+ur~) = sum_e alpha*ul~ + ur~, recovered via 1/|att|.
"""
import numpy as np
import ml_dtypes

try:
    import concourse  # noqa: F401
except ImportError:  # pragma: no cover
    import sys
    sys.path.insert(0, "/opt/trn_rl_repo")

from concourse import bass, bacc, mybir, tile
from concourse import bass_utils

F32 = mybir.dt.float32
BF16 = mybir.dt.bfloat16
I16 = mybir.dt.int16
NPBF = ml_dtypes.bfloat16

N_NODES = 50000
N_CORES = 8
FEAT = 128
HEADS1 = 4


class Cfg:
    def __init__(self, n_nodes, n_cores, feat, heads1):
        self.N = n_nodes
        self.NC = n_cores
        self.NPC = n_nodes // n_cores
        self.P = 128
        self.CHUNKS = (self.NPC + 127) // 128
        self.SLOTS = self.CHUNKS * 128
        self.TOT = self.SLOTS * n_cores      # rows in the gathered table
        self.F = feat
        self.H1 = heads1
        self.T = None
        self.TD = BF16
        self.queues = 4
        self.repeats = 1
        self.host_onehots = False


# ---------------------------------------------------------------- host prep

def prep_weights(att, Wl, bl, Wr, br, bias):
    a = att.reshape(-1).astype(np.float64)
    absa = np.maximum(np.abs(a), 1e-12)
    sign = np.where(a >= 0, 1.0, -1.0)
    return dict(
        Wl=(Wl * absa[None, :]).astype(np.float32),
        bl=(bl * absa).astype(np.float32),
        Wr=(Wr * absa[None, :]).astype(np.float32),
        br=(br * absa).astype(np.float32),
        inva=(1.0 / absa).astype(np.float32),
        sign=sign.astype(np.float32),
        bias=bias.astype(np.float32),
    )


def prep_graph(edge_index, cfg):
    """Slot permutation + per-chunk edge layout for dma_gather (int16 idx).

    Nodes of each core are bin-packed into CHUNKS bins of <=128 slots,
    balancing edges per bin; slot = (bin, lane). Edges are placed on their
    dst core/chunk, split into [src_newid < 32768 | >= 32768] sections,
    each padded to global tile counts T_LO / T_HI. newid = core*SLOTS+slot.
    """
    import heapq
    N, NPC, P, CHUNKS, SLOTS, NC = (cfg.N, cfg.NPC, cfg.P, cfg.CHUNKS,
                                    cfg.SLOTS, cfg.NC)
    HALF = 32768
    src = np.asarray(edge_index[0], dtype=np.int64)
    dst = np.asarray(edge_index[1], dtype=np.int64)
    loops = np.arange(N, dtype=np.int64)
    src = np.concatenate([src, loops])
    dst = np.concatenate([dst, loops])

    # pass 1: slot assignment per core
    newid = np.full(N, -1, dtype=np.int64)
    node_ids_all = []
    per_core_edges = []
    for c in range(NC):
        lo = c * NPC
        m = (dst >= lo) & (dst < lo + NPC)
        s_c = src[m]
        d_c = dst[m] - lo
        per_core_edges.append((s_c, d_c))
        deg = np.bincount(d_c, minlength=NPC)
        order = np.argsort(-deg, kind="stable")
        heap = [(0, g) for g in range(CHUNKS)]
        heapq.heapify(heap)
        bin_cnt = [0] * CHUNKS
        bin_sum = [0] * CHUNKS
        node_ids = np.full((CHUNKS, P), -1, dtype=np.int64)
        for n in order:
            while True:
                sm, g = heapq.heappop(heap)
                if bin_cnt[g] < P:
                    break
            node_ids[g, bin_cnt[g]] = n
            newid[lo + n] = c * SLOTS + g * P + bin_cnt[g]
            bin_cnt[g] += 1
            bin_sum[g] = sm + int(deg[n])
            if bin_cnt[g] < P:
                heapq.heappush(heap, (bin_sum[g], g))
        node_ids_all.append(node_ids)

    # pass 2: per-chunk edge sections with src newids
    cores_chunk_edges = []
    maxTlo = maxThi = 0
    for c in range(NC):
        s_c, d_c = per_core_edges[c]
        sid = newid[s_c]                      # src new global id
        dslot = newid[c * NPC + d_c] - c * SLOTS  # local slot in [0, SLOTS)
        g_of = dslot // P
        chunk_edges = []
        for g in range(CHUNKS):
            m = g_of == g
            sg = sid[m]
            tg = dslot[m] - g * P             # lane 0..127
            lo_m = sg < HALF
            lo_s, lo_t = sg[lo_m], tg[lo_m]
            hi_s, hi_t = sg[~lo_m] - HALF, tg[~lo_m]
            maxTlo = max(maxTlo, (len(lo_s) + P - 1) // P)
            maxThi = max(maxThi, (len(hi_s) + P - 1) // P)
            chunk_edges.append((lo_s, lo_t, hi_s, hi_t))
        cores_chunk_edges.append(chunk_edges)

    T_LO = max(maxTlo, 1)
    T_HI = maxThi if cfg.TOT > HALF else 0
    if cfg.TOT > HALF:
        T_HI = max(T_HI, 1)
    T = T_LO + T_HI

    def wrap16(ids):
        a = np.asarray(ids, dtype=np.int16).reshape(-1, 16).T
        return np.tile(a, (8, 1))

    out = []
    for c in range(NC):
        chunk_edges = cores_chunk_edges[c]
        xlw = np.zeros((CHUNKS, P, T * 8), dtype=np.int16)
        dstl = np.full((CHUNKS, P, T), 999.0, dtype=NPBF)
        dstlT = np.full((CHUNKS, T * P), 999.0, dtype=NPBF)
        for g in range(CHUNKS):
            lo_s, lo_t, hi_s, hi_t = chunk_edges[g]
            n_lo, n_hi = T_LO * P, T_HI * P
            ls = np.zeros(n_lo, np.int64); ls[:len(lo_s)] = lo_s
            sl = np.full(n_lo + n_hi, 999.0, np.float32)
            sl[:len(lo_t)] = lo_t
            xlw[g, :, :T_LO * 8] = wrap16(ls)
            if T_HI > 0:
                hs = np.zeros(n_hi, np.int64); hs[:len(hi_s)] = hi_s
                sl[n_lo:n_lo + len(hi_t)] = hi_t
                xlw[g, :, T_LO * 8:] = wrap16(hs)
            # edge i -> (t = i//128, lane = i%128)
            dstl[g] = sl.reshape(T, P).T.astype(NPBF)
            dstlT[g] = sl.astype(NPBF)
        gr = dict(xlw=xlw, dstl=dstl, dstlT=dstlT,
                  node_ids=node_ids_all[c])
        if getattr(cfg, "host_onehots", False):
            ohde = np.zeros((CHUNKS, P, T * P), dtype=NPBF)
            ohag = np.zeros((CHUNKS, P, T * P), dtype=NPBF)
            for g in range(CHUNKS):
                sl = dstlT[g].astype(np.float32)
                pos = np.arange(T * P)
                valid = sl < P
                s_i = sl[valid].astype(np.int64)
                p_i = pos[valid]
                # ohde[p, (t,j)] = [slot(edge(t,j)) == p]
                ohde[g][s_i, p_i] = 1.0
                # ohag[p, (t,j)] = [slot(edge(t,p)) == j]
                lane = p_i % P
                col = (p_i // P) * P + s_i
                ohag[g][lane, col] = 1.0
            gr["ohde"] = ohde
            gr["ohag"] = ohag
        out.append(gr)
    return out, (T, T_LO, T_HI)


def make_core_inputs(core_id, x, w1, w2, gr, cfg):
    SLOTS, F, P = cfg.SLOTS, cfg.F, cfg.P
    nid = gr["node_ids"].ravel()
    xb = np.zeros((SLOTS, F), np.float32)
    valid = nid >= 0
    xb[valid] = x[core_id * cfg.NPC + nid[valid]]
    rowb = lambda v: np.broadcast_to(v.astype(np.float32), (P, F)).copy()
    rowb16 = lambda v: np.broadcast_to(v.astype(NPBF), (P, F)).copy()
    return {
        "xT_own": np.ascontiguousarray(xb.T).astype(NPBF),
        "W1l": w1["Wl"].astype(NPBF), "W1r": w1["Wr"].astype(NPBF),
        "W2l": w2["Wl"].astype(NPBF), "W2r": w2["Wr"].astype(NPBF),
        "bb1l": rowb(w1["bl"]), "bb1r": rowb(w1["br"]),
        "bb2l": rowb(w2["bl"]), "bb2r": rowb(w2["br"]),
        "inva1": rowb(w1["inva"]), "gbias1": rowb(w1["bias"]),
        "inva2": rowb(w2["inva"]), "gbias2": rowb(w2["bias"]),
        "sgn1": rowb16(w1["sign"]), "sgn2": rowb16(w2["sign"]),
        "identb": np.eye(P, dtype=NPBF),
        "iotac": np.arange(P, dtype=np.float32).reshape(P, 1),
        "iotab": np.broadcast_to(np.arange(P, dtype=NPBF), (P, P)).copy(),
        "xlw": gr["xlw"], "dstl": gr["dstl"], "dstlT": gr["dstlT"],
    } | ({"ohde": gr["ohde"], "ohag": gr["ohag"]}
         if getattr(cfg, "host_onehots", False) else {})


# ---------------------------------------------------------------- device

def declare_io(nc, cfg):
    CH, P, T, F, SLOTS = cfg.CHUNKS, cfg.P, cfg.T, cfg.F, cfg.SLOTS
    TD = cfg.TD
    d = {}
    def inp(name, shape, dt):
        d[name] = nc.dram_tensor(name, list(shape), dt, kind="ExternalInput").ap()
    inp("xT_own", (F, SLOTS), TD)
    for n in ("W1l", "W1r", "W2l", "W2r", "sgn1", "sgn2", "iotab"):
        inp(n, (P, F), TD)
    for n in ("bb1l", "bb1r", "bb2l", "bb2r",
              "inva1", "gbias1", "inva2", "gbias2"):
        inp(n, (P, F), F32)
    inp("identb", (P, P), TD)
    inp("iotac", (P, 1), F32)
    inp("xlw", (CH, P, T * 8), I16)
    inp("dstl", (CH, P, T), TD)
    inp("dstlT", (CH, T * P), TD)
    if getattr(cfg, "host_onehots", False):
        inp("ohde", (CH, P, T * P), TD)
        inp("ohag", (CH, P, T * P), TD)
    d["out"] = nc.dram_tensor("out", [SLOTS, F], F32, kind="ExternalOutput").ap()
    return d


def build_program(tc, io, cfg):
    nc = tc.nc
    P, F, T, CH = cfg.P, cfg.F, cfg.T, cfg.CHUNKS
    SLOTS, TD, TOT = cfg.SLOTS, cfg.TD, cfg.TOT
    TLO, THI = cfg.T_LO, cfg.T_HI
    HALF = 32768
    H1 = cfg.H1
    MAXT = 8
    qctr = [0]

    with (
        tc.tile_pool(name="consts", bufs=1) as cpool,
        tc.tile_pool(name="work", bufs=getattr(cfg, "wp_bufs", 3)) as wp,
        tc.tile_pool(name="small", bufs=getattr(cfg, "sp_bufs", 3)) as sp,
        tc.tile_pool(name="psum", bufs=getattr(cfg, "pp_bufs", 2),
                     space="PSUM") as pp,
        tc.tile_pool(name="dram", bufs=1, space="DRAM") as dp,
    ):
        C = {}
        for n in ("W1l", "W1r", "W2l", "W2r", "sgn1", "sgn2", "iotab"):
            t = cpool.tile([P, F], TD, tag=n)
            nc.sync.dma_start(t[:], io[n])
            C[n] = t
        for n in ("bb1l", "bb1r", "bb2l", "bb2r",
                  "inva1", "gbias1", "inva2", "gbias2"):
            t = cpool.tile([P, F], F32, tag=n)
            nc.sync.dma_start(t[:], io[n])
            C[n] = t
        identb = cpool.tile([P, P], TD, tag="identb")
        nc.sync.dma_start(identb[:], io["identb"])
        iotac = cpool.tile([P, 1], F32, tag="iotac")
        nc.sync.dma_start(iotac[:], io["iotac"])

        xl_own = dp.tile([SLOTS, F], TD)
        xr_own = dp.tile([SLOTS, F], TD)
        h_block = dp.tile([SLOTS, F], TD)
        hl_own = dp.tile([SLOTS, F], TD)
        hr_own = dp.tile([SLOTS, F], TD)
        ag_space = "Shared" if cfg.NC > 1 else "Local"

        def table_phase(src_rows, Wl, Wr, bbl, bbr, dst_l, dst_r, transpose):
            for g in range(CH):
                xT_sb = sp.tile([P, P], TD, tag="xT")
                if transpose:
                    h_sb = sp.tile([P, P], TD, tag="h_sb")
                    nc.sync.dma_start(h_sb[:], src_rows[g * P:(g + 1) * P, :])
                    ps_t = pp.tile([P, P], TD, tag="pst")
                    nc.tensor.transpose(out=ps_t[:], in_=h_sb[:],
                                        identity=identb[:])
                    nc.vector.tensor_copy(out=xT_sb[:], in_=ps_t[:])
                else:
                    nc.sync.dma_start(xT_sb[:], src_rows[:, g * P:(g + 1) * P])
                ps_l = pp.tile([P, F], F32, tag="agg")
                ps_r = pp.tile([P, F], F32, tag="psg")
                nc.tensor.matmul(ps_l[:], lhsT=xT_sb[:], rhs=Wl[:],
                                 start=True, stop=True)
                nc.tensor.matmul(ps_r[:], lhsT=xT_sb[:], rhs=Wr[:],
                                 start=True, stop=True)
                xl_sb = sp.tile([P, F], TD, tag="xl_sb")
                xr_sb = sp.tile([P, F], TD, tag="xr_sb")
                nc.vector.tensor_tensor(out=xl_sb[:], in0=ps_l[:], in1=bbl[:],
                                        op=mybir.AluOpType.add)
                nc.vector.tensor_tensor(out=xr_sb[:], in0=ps_r[:], in1=bbr[:],
                                        op=mybir.AluOpType.add)
                nc.sync.dma_start(dst_l[g * P:(g + 1) * P, :], xl_sb[:])
                nc.sync.dma_start(dst_r[g * P:(g + 1) * P, :], xr_sb[:])

        def all_gather(own, full):
            if cfg.NC == 1:
                nc.sync.dma_start(full[:, :], own[0:SLOTS, :])
            else:
                nc.gpsimd.collective_compute(
                    "AllGather", mybir.AluOpType.bypass,
                    replica_groups=[list(range(cfg.NC))],
                    ins=[own[0:SLOTS, :]], outs=[full[:, :]],
                )

        def edge_layer(tab_full, tab_own, H, sgn, inva, gbias, elu, out_to):
            Ch = F // H
            NG = (T + 3) // 4                      # 4-tile PSUM groups
            for g in range(CH):
                xlw_sb = sp.tile([P, T * 8], I16, tag="xlw")
                nc.sync.dma_start(xlw_sb[:], io["xlw"][g])
                urt = sp.tile([P, F], TD, tag="urt")
                nc.sync.dma_start(urt[:], tab_own[g * P:(g + 1) * P, :])
                if not cfg.host_onehots:
                    dstl_sb = sp.tile([P, T], TD, tag="dstl")
                    nc.sync.dma_start(dstl_sb[:], io["dstl"][g])
                    dstb = wp.tile([P, T * P], TD, tag="dstb")
                    if getattr(cfg, "no_dstb", False):
                        nc.vector.memset(dstb[:], 999.0)
                    else:
                        nc.sync.dma_start(
                            dstb[:],
                            io["dstlT"][g:g + 1, :].to_broadcast([P, T * P]))

                ul = wp.tile([P, T * F], TD, tag="ul")
                ul3 = ul[:].rearrange("p (t f) -> p t f", f=F)
                if getattr(cfg, "no_ul", False):
                    nc.vector.memset(ul[:], 0.0)
                for a in ([] if getattr(cfg, "no_ul", False)
                          else range(0, TLO, MAXT)):
                    b = min(a + MAXT, TLO)
                    nc.gpsimd.dma_gather(
                        out_ap=ul3[:, a:b, :], in_ap=tab_full[0:min(HALF, TOT), :],
                        idxs_ap=xlw_sb[:, a * 8:b * 8],
                        num_idxs=(b - a) * P, num_idxs_reg=(b - a) * P,
                        elem_size=F, queue_num=qctr[0] % cfg.queues,
                        single_packet=True)
                    qctr[0] += 1
                for a in ([] if getattr(cfg, "no_ul", False)
                          else range(TLO, T, MAXT)):
                    b = min(a + MAXT, T)
                    nc.gpsimd.dma_gather(
                        out_ap=ul3[:, a:b, :], in_ap=tab_full[HALF:TOT, :],
                        idxs_ap=xlw_sb[:, a * 8:b * 8],
                        num_idxs=(b - a) * P, num_idxs_reg=(b - a) * P,
                        elem_size=F, queue_num=qctr[0] % cfg.queues,
                        single_packet=True)
                    qctr[0] += 1

                oh_de = wp.tile([P, T * P], TD, tag="oh_de")
                oh_ag = wp.tile([P, T * P], TD, tag="oh_ag")
                if cfg.host_onehots:
                    nc.sync.dma_start(oh_de[:], io["ohde"][g])
                    nc.sync.dma_start(oh_ag[:], io["ohag"][g])
                else:
                    # one-hot builds: 2 DVE ops for the whole chunk
                    nc.vector.tensor_scalar(
                        out=oh_de[:], in0=dstb[:], scalar1=iotac[:, 0:1],
                        scalar2=None, op0=mybir.AluOpType.is_equal)
                    nc.vector.tensor_tensor(
                        out=oh_ag[:].rearrange("p (t f) -> p t f", f=P),
                        in0=C["iotab"][:].rearrange("p (o f) -> p o f", o=1)
                            .to_broadcast([P, T, P]),
                        in1=dstl_sb[:].rearrange("p (t o) -> p t o", o=1)
                            .to_broadcast([P, T, P]),
                        op=mybir.AluOpType.is_equal)
                oh_de3 = oh_de[:].rearrange("p (t f) -> p t f", f=P)
                oh_ag3 = oh_ag[:].rearrange("p (t f) -> p t f", f=P)

                # u~ = ul[src] + ur[dst] summed in PSUM; lrelu from PSUM
                lr = wp.tile([P, T * F], TD, tag="lr")
                for grp in ([] if getattr(cfg, "no_mm", False) else range(NG)):
                    t0, t1 = grp * 4, min(grp * 4 + 4, T)
                    ncols = (t1 - t0) * F
                    psg = pp.tile([P, 4 * F], F32, tag="psg")
                    for t in range(t0, t1):
                        c0 = (t - t0) * F
                        nc.tensor.matmul(psg[:, c0:c0 + F], lhsT=oh_de3[:, t, :],
                                         rhs=urt[:], start=True, stop=False)
                        nc.tensor.matmul(psg[:, c0:c0 + F], lhsT=identb[:],
                                         rhs=ul3[:, t, :], start=False, stop=True)
                    if getattr(cfg, "sim_safe", False):
                        t02 = sp.tile([P, 4 * F], F32, tag="t02")
                        nc.vector.tensor_scalar(
                            out=t02[:, 0:ncols], in0=psg[:, 0:ncols],
                            scalar1=0.2, scalar2=None,
                            op0=mybir.AluOpType.mult)
                        nc.vector.tensor_tensor(
                            out=lr[:, t0 * F:t0 * F + ncols],
                            in0=psg[:, 0:ncols], in1=t02[:, 0:ncols],
                            op=mybir.AluOpType.max)
                    else:
                        nc.scalar.activation(
                            out=lr[:, t0 * F:t0 * F + ncols], in_=psg[:, 0:ncols],
                            func=mybir.ActivationFunctionType.Prelu, alpha=0.2)

                # logits = reduce(sign * lr) per (tile, head)
                sgt = wp.tile([P, T * F], TD, tag="sgt")
                nc.vector.tensor_tensor(
                    out=sgt[:].rearrange("p (t f) -> p t f", f=F),
                    in0=lr[:].rearrange("p (t f) -> p t f", f=F),
                    in1=sgn[:].rearrange("p (o f) -> p o f", o=1)
                        .to_broadcast([P, T, F]),
                    op=mybir.AluOpType.mult)
                logit = sp.tile([P, T * H], F32, tag="logit")
                nc.vector.tensor_reduce(
                    out=logit[:].rearrange("p (t h o) -> p t h o", h=H, o=1),
                    in_=sgt[:].rearrange("p (t h c) -> p t h c", h=H, c=Ch),
                    axis=mybir.AxisListType.X, op=mybir.AluOpType.add)

                aug = wp.tile([P, T * (F + H)], TD, tag="aug")
                aug3 = aug[:].rearrange("p (t c) -> p t c", c=F + H)
                nc.scalar.activation(out=aug3[:, :, F:F + H], in_=logit[:],
                                     func=mybir.ActivationFunctionType.Exp)
                # aggregate alpha*ul directly (sum alpha = 1 per dst, and
                # GATv2 aggregates xl[src] only — no ur term to remove)
                ul4 = ul[:].rearrange("p (t h c) -> p t h c", h=H, c=Ch)
                aug4 = aug3[:, :, 0:F].rearrange("p t (h c) -> p t h c", h=H)
                wb = aug3[:, :, F:F + H].to_broadcast([P, T, H, Ch])
                nc.vector.tensor_tensor(out=aug4, in0=ul4, in1=wb,
                                        op=mybir.AluOpType.mult)

                ps = pp.tile([P, F + H], F32, tag="agg")
                TAGG = 1 if getattr(cfg, "no_agg", False) else T
                for t in range(TAGG):
                    nc.tensor.matmul(ps[:], lhsT=oh_ag3[:, t, :],
                                     rhs=aug3[:, t, :],
                                     start=(t == 0), stop=(t == TAGG - 1))

                den = sp.tile([P, H], F32, tag="den")
                nc.vector.tensor_scalar(out=den[:], in0=ps[:, F:F + H],
                                        scalar1=1e-30, scalar2=None,
                                        op0=mybir.AluOpType.add)
                rec = sp.tile([P, H], F32, tag="rec")
                nc.vector.reciprocal(rec[:], den[:])
                o1 = sp.tile([P, F], F32, tag="o1")
                if H > 1:
                    nc.vector.tensor_tensor(
                        out=o1[:].rearrange("p (h c) -> p h c", h=H),
                        in0=ps[:, 0:F].rearrange("p (h c) -> p h c", h=H),
                        in1=rec[:].rearrange("p (h o) -> p h o", o=1)
                            .to_broadcast([P, H, Ch]),
                        op=mybir.AluOpType.mult)
                else:
                    nc.vector.tensor_scalar(out=o1[:], in0=ps[:, 0:F],
                                            scalar1=rec[:, 0:1], scalar2=None,
                                            op0=mybir.AluOpType.mult)
                nc.vector.tensor_tensor(out=o1[:], in0=o1[:], in1=inva[:],
                                        op=mybir.AluOpType.mult)
                nc.vector.tensor_tensor(out=o1[:], in0=o1[:], in1=gbias[:],
                                        op=mybir.AluOpType.add)
                if elu:
                    m0 = sp.tile([P, F], F32, tag="m0")
                    nc.vector.tensor_scalar(out=m0[:], in0=o1[:], scalar1=0.0,
                                            scalar2=None, op0=mybir.AluOpType.min)
                    e0 = sp.tile([P, F], F32, tag="e0")
                    nc.scalar.activation(out=e0[:], in_=m0[:],
                                         func=mybir.ActivationFunctionType.Exp)
                    nc.vector.tensor_scalar(out=o1[:], in0=o1[:], scalar1=0.0,
                                            scalar2=None, op0=mybir.AluOpType.max)
                    nc.vector.tensor_tensor(out=o1[:], in0=o1[:], in1=e0[:],
                                            op=mybir.AluOpType.add)
                    nc.vector.tensor_scalar(out=o1[:], in0=o1[:], scalar1=1.0,
                                            scalar2=None,
                                            op0=mybir.AluOpType.subtract)
                if out_to is h_block:
                    hcast = sp.tile([P, F], TD, tag="hcast")
                    nc.vector.tensor_copy(out=hcast[:], in_=o1[:])
                    nc.sync.dma_start(out_to[g * P:(g + 1) * P, :], hcast[:])
                else:
                    nc.sync.dma_start(out_to[g * P:(g + 1) * P, :], o1[:])

        for _rep in range(cfg.repeats):
            xl_full = dp.tile([TOT, F], TD, addr_space=ag_space)
            hl_full = dp.tile([TOT, F], TD, addr_space=ag_space)
            table_phase(io["xT_own"], C["W1l"], C["W1r"], C["bb1l"], C["bb1r"],
                        xl_own, xr_own, transpose=False)
            all_gather(xl_own, xl_full)
            edge_layer(xl_full, xr_own, H1, C["sgn1"], C["inva1"], C["gbias1"],
                       elu=True, out_to=h_block)
            table_phase(h_block, C["W2l"], C["W2r"], C["bb2l"], C["bb2r"],
                        hl_own, hr_own, transpose=True)
            all_gather(hl_own, hl_full)
            edge_layer(hl_full, hr_own, 1, C["sgn2"], C["inva2"], C["gbias2"],
                       elu=False, out_to=io["out"])


# ---------------------------------------------------------------- runner

_LAST = {}


def _build(inputs, cfg):
    x = np.asarray(inputs["x"], np.float32)
    ei = np.asarray(inputs["edge_index"])
    w1 = prep_weights(np.asarray(inputs["att1"], np.float32),
                      np.asarray(inputs["W1l"], np.float32),
                      np.asarray(inputs["b1l"], np.float32),
                      np.asarray(inputs["W1r"], np.float32),
                      np.asarray(inputs["b1r"], np.float32),
                      np.asarray(inputs["bias1"], np.float32))
    w2 = prep_weights(np.asarray(inputs["att2"], np.float32),
                      np.asarray(inputs["W2l"], np.float32),
                      np.asarray(inputs["b2l"], np.float32),
                      np.asarray(inputs["W2r"], np.float32),
                      np.asarray(inputs["b2r"], np.float32),
                      np.asarray(inputs["bias2"], np.float32))
    grs, (T, T_LO, T_HI) = prep_graph(ei, cfg)
    cfg.T, cfg.T_LO, cfg.T_HI = T, T_LO, T_HI
    in_maps = [make_core_inputs(c, x, w1, w2, grs[c], cfg)
               for c in range(cfg.NC)]
    nc = bacc.Bacc("TRN2", target_bir_lowering=False, debug=False,
                   num_devices=cfg.NC, num_swdge_queues=cfg.queues)
    io = declare_io(nc, cfg)
    with tile.TileContext(nc) as tc:
        build_program(tc, io, cfg)
    nc.compile()
    return nc, in_maps, grs


def kernel(**inputs) -> np.ndarray:
    cfg = Cfg(N_NODES, N_CORES, FEAT, HEADS1)
    nc, in_maps, grs = _build(inputs, cfg)
    res = bass_utils.run_bass_kernel_spmd(nc, in_maps,
                                          core_ids=list(range(cfg.NC)))
    _LAST.update(results=res, nc=nc, in_maps=in_maps, cfg=cfg, grs=grs)

    out = np.zeros((cfg.N, cfg.F), np.float32)
    for c in range(cfg.NC):
        oc = np.asarray(res.results[c]["out"]).reshape(cfg.SLOTS, cfg.F)
        nid = grs[c]["node_ids"].ravel()
        valid = nid >= 0
        out[c * cfg.NPC + nid[valid]] = oc[valid]
    return out
